# revision 12
# baseline (speedup 1.0000x reference)
"""Trainium2 Bass kernel for y[b,o] = sum_k w[o,k] * x[b, idx[o,k]].

B=32, N_IN=1e6, N_OUT=5e5, K=3.

Batch-pair packing: two batch rows are packed as (fp16, fp16) into each
32-bit SBUF lane, so one gather index moves all 32 batch values. Cores are
output-sharded 8 ways (62.5k outputs each), every core sees the full
packed x. gpsimd ap_gather cost is per-index (~27ns), so halving the
index count per core halves both gather stages vs the batch-split layout.

Per-core pipeline (device, all values are fp16 pairs in fp32 lanes):
  Stage 1: x split into 64 windows of 16384 dofs; 8 windows in flight on
    the 8 gpsimd cores. ap_gather pulls contributions bin-padded by
    (window, output-sub) into an HBM buffer C.
  Stage 2: per output-sub, the 64 window-bins are loaded from C,
    ap_gather reorders to (o, k) order, VectorE multiplies by fp16
    weights and reduces K=3 (both on the fp16 element view), and yt
    streams to y as fp32 pairs.

Host packs x (fp32 -> fp16 pairs), bins/balances indices, and unpacks y.
"""
import numpy as np

B = 32
N_IN = 1_000_000
N_OUT = 500_000
K = 3

N_IN_PAD = 1 << 20       # 128 windows * 8192
WIN = 8192               # dofs per window
WSH = 13                 # log2(WIN)
NW = 128                 # windows
NR = 16                  # stage-1 rounds (8 windows in flight)
NO_CORE = 62_720         # outputs per core (62500 real + 220 pad)
NO_REAL = 62_500
NS = 40                  # subs
SUB = 1568               # outputs per sub (40*1568 = 62720)
S2R = 5                  # stage-2 rounds (8 subs in flight)
NI2 = SUB * K            # stage-2 idxs per sub = 4704

_CACHE = {}


def _ceil_to(x, m):
    return (x + m - 1) // m * m


def _wrap16(a2):
    """[..., n] -> [..., 16, n//16]: partition j holds a[j::16]."""
    sh = a2.shape[:-1]
    n = a2.shape[-1]
    return np.ascontiguousarray(
        a2.reshape(*sh, n // 16, 16).swapaxes(-1, -2)
    )


def _balance_subs(idx_c):
    """Greedy assignment of outputs to subs, balancing (window, sub) bins."""
    no = idx_c.shape[0]
    wid3 = (idx_c.astype(np.int64) >> WSH)          # [no, K]
    rng = np.random.default_rng(1234)
    order = rng.permutation(no)
    cnt = np.zeros((NW, NS), np.int32)
    fill = np.zeros(NS, np.int32)
    assign = np.empty(no, np.int32)
    big = np.int32(1 << 20)
    for o in order:
        w3 = wid3[o]
        load = cnt[w3].max(axis=0) + (fill >= SUB) * big
        s = int(np.argmin(load))
        assign[o] = s
        cnt[w3, s] += 1
        fill[s] += 1
    return assign


def _bin_sizes(idx_c, assign):
    dof = idx_c.reshape(-1).astype(np.int64)
    wid = dof >> WSH
    sub = np.repeat(assign.astype(np.int64), K)
    return np.bincount(wid * NS + sub, minlength=NW * NS).reshape(NW, NS)


def _prep_core(idx_c, w_c, caps, coff, assign):
    """Host-side binning for one core given per-window bin capacities."""
    no = idx_c.shape[0]
    dof = idx_c.reshape(-1).astype(np.int64)          # [no*K], (o,k) order
    wid = dof >> WSH
    loc = (dof & (WIN - 1)).astype(np.int64)
    sub = np.repeat(assign.astype(np.int64), K)

    binid = wid * NS + sub
    order = np.lexsort((np.arange(dof.size), binid))
    bin_sizes = np.bincount(binid, minlength=NW * NS)
    bin_starts = np.concatenate([[0], np.cumsum(bin_sizes)])
    rank = np.empty(dof.size, dtype=np.int64)
    rank[order] = np.arange(dof.size) - bin_starts[binid[order]]

    # stage-1 idx lists: window w's list is [NS, caps[w]] with bin (w,s)
    # occupying the first n(w,s) columns of row s.
    ni1 = NS * int(caps.max())
    s1 = np.tile((np.arange(ni1, dtype=np.int64) * 97 % WIN).astype(np.int16), (NW, 1))
    within = sub * caps[wid] + rank
    s1[wid, within] = loc.astype(np.int16)

    # stage-2 slots (natural (o,k) order): csub col = coff[w] + rank
    slots = (coff[wid] + rank).astype(np.int16)

    s2i = np.zeros((S2R, 8, NI2), dtype=np.int16)
    wrep = np.zeros((S2R, 8, NI2), dtype=np.float32)
    w_flat = w_c.reshape(-1).astype(np.float32)
    slots3 = slots.reshape(no, K)
    w3 = w_flat.reshape(no, K)
    outs_of_sub = []
    for s in range(NS):
        r2, u = divmod(s, 8)
        outs = np.where(assign == s)[0]
        outs_of_sub.append(outs)
        m = outs.size * K
        s2i[r2, u, :m] = slots3[outs].reshape(-1)
        wrep[r2, u, :m] = w3[outs].reshape(-1)
    return {"s1": s1, "s2i": s2i, "wrep": wrep, "outs_of_sub": outs_of_sub}


def _build_nc(caps, coff, cw):
    import concourse.bacc as bacc
    import concourse.tile as tile
    import concourse.mybir as mybir

    ni1 = NS * int(caps.max())   # stage-1 num_idxs per round

    nc = bacc.Bacc("TRN2", target_bir_lowering=False, debug=False, num_devices=8)
    xg_d = nc.dram_tensor("xg", [16, N_IN_PAD], mybir.dt.float32, kind="ExternalInput")
    s1i_d = nc.dram_tensor("s1i", [NR, 128, ni1 // 16], mybir.dt.int16, kind="ExternalInput")
    s2i_d = nc.dram_tensor("s2i", [S2R, 128, NI2 // 16], mybir.dt.int16, kind="ExternalInput")
    wr_d = nc.dram_tensor("wr", [S2R, 128, NI2 * 2], mybir.dt.float16, kind="ExternalInput")
    y_d = nc.dram_tensor("y", [16, NS * SUB], mybir.dt.float32, kind="ExternalOutput")
    c_d = nc.dram_tensor("cbuf", [16, NS, cw], mybir.dt.float32)

    with tile.TileContext(nc) as tc:
      with tc.tile_pool(name="px", bufs=3) as px, \
           tc.tile_pool(name="p1", bufs=2) as p1:
        # tiny dummy gather: triggers the gpsimd ext-isa library IRAM load
        # so it overlaps the first x-window DMA instead of serializing.
        dum_in = p1.tile([128, 16], mybir.dt.float32)
        dum_idx = p1.tile([128, 1], mybir.dt.int16)
        dum_out = p1.tile([128, 16], mybir.dt.float32)
        nc.vector.memset(dum_in[:], 0.0)
        nc.vector.memset(dum_idx[:], 0)
        nc.gpsimd.ap_gather(
            out_ap=dum_out[:].rearrange("p (n d) -> p n d", d=1),
            in_ap=dum_in[:].rearrange("p (n d) -> p n d", d=1),
            idxs_ap=dum_idx[:],
            channels=128,
            num_elems=16,
            d=1,
            num_idxs=16,
        )
        for r in range(NR):
            xwin = px.tile([128, WIN], mybir.dt.float32)
            nc.sync.dma_start(
                xwin[:],
                xg_d.ap()[:, r * 8 * WIN : (r + 1) * 8 * WIN].rearrange(
                    "b (u j) -> u b j", u=8
                ),
            )
            s1idx = p1.tile([128, ni1 // 16], mybir.dt.int16)
            nc.sync.dma_start(s1idx[:], s1i_d.ap()[r])
            g1 = p1.tile([128, ni1], mybir.dt.float32)
            nc.gpsimd.ap_gather(
                out_ap=g1[:].rearrange("p (n d) -> p n d", d=1),
                in_ap=xwin[:].rearrange("p (n d) -> p n d", d=1),
                idxs_ap=s1idx[:],
                channels=128,
                num_elems=WIN,
                d=1,
                num_idxs=ni1,
            )
            pb = int(caps[0])
            dst = c_d.ap()[:, :, r * 8 * pb : (r + 1) * 8 * pb].rearrange(
                "b s (u j) -> u b s j", u=8
            )
            nc.scalar.dma_start(dst, g1[:])

      with tc.tile_pool(name="pc", bufs=2) as pc, \
           tc.tile_pool(name="p2", bufs=2) as p2:
        for r2 in range(S2R):
            csub = pc.tile([128, cw], mybir.dt.float32)
            nc.sync.dma_start(
                csub[:],
                c_d.ap()[:, r2 * 8 : (r2 + 1) * 8, :].rearrange("b u j -> u b j"),
            )
            s2idx = p2.tile([128, NI2 // 16], mybir.dt.int16)
            nc.scalar.dma_start(s2idx[:], s2i_d.ap()[r2])
            wt = p2.tile([128, NI2 * 2], mybir.dt.float16)
            nc.scalar.dma_start(wt[:], wr_d.ap()[r2])
            g2 = p2.tile([128, NI2], mybir.dt.float32)
            nc.gpsimd.ap_gather(
                out_ap=g2[:].rearrange("p (n d) -> p n d", d=1),
                in_ap=csub[:].rearrange("p (n d) -> p n d", d=1),
                idxs_ap=s2idx[:],
                channels=128,
                num_elems=cw,
                d=1,
                num_idxs=NI2,
            )
            g2h = g2[:].bitcast(mybir.dt.float16)           # [128, NI2*2]
            nc.vector.tensor_tensor(
                out=g2h, in0=g2h, in1=wt[:], op=mybir.AluOpType.mult
            )
            yt = p2.tile([128, SUB * 2], mybir.dt.float16)
            with nc.allow_low_precision(reason="fp16 pair k-sum of 3 terms"):
                nc.vector.tensor_reduce(
                    out=yt[:].rearrange("p (o pr) -> p o pr", pr=2),
                    in_=g2h.rearrange("p (o k pr) -> p o pr k", k=K, pr=2),
                    axis=mybir.AxisListType.X,
                    op=mybir.AluOpType.add,
                )
            nc.scalar.dma_start(
                y_d.ap()[:, r2 * 8 * SUB : (r2 + 1) * 8 * SUB].rearrange(
                    "b (u o) -> u b o", u=8
                ),
                yt[:].bitcast(mybir.dt.float32),
            )
    nc.compile()
    return nc


def _pack_x_pairs(x):
    """x [32, N_IN] f32 -> packed [16, N_IN_PAD] f32: lane l holds
    (fp16 x[2l, d], fp16 x[2l+1, d]) in each 32-bit element."""
    xh = x.astype(np.float16)                        # [32, N_IN]
    pk = np.zeros((16, N_IN_PAD, 2), dtype=np.float16)
    pk[:, :N_IN, 0] = xh[0::2]
    pk[:, :N_IN, 1] = xh[1::2]
    return pk.reshape(16, N_IN_PAD * 2).view(np.float32)


def kernel(x, w, idx):
    from concourse.bass_utils import run_bass_kernel_spmd

    x = np.asarray(x, dtype=np.float32)
    w = np.asarray(w, dtype=np.float32)
    idx = np.asarray(idx)
    xpk = _pack_x_pairs(x)

    # 8-way output shard, padded with dummy outputs (w 0, idx spread across
    # windows so the pad contributions don't skew any single bin)
    idx_pad = np.zeros((8 * NO_CORE, K), dtype=np.int32)
    w_pad = np.zeros((8 * NO_CORE, K), dtype=np.float32)
    npad = NO_CORE - NO_REAL
    spread = (np.arange(npad * K, dtype=np.int64) * 104729) % N_IN
    for c in range(8):
        idx_pad[c * NO_CORE : c * NO_CORE + NO_REAL] = idx[c * NO_REAL : (c + 1) * NO_REAL]
        idx_pad[c * NO_CORE + NO_REAL : (c + 1) * NO_CORE] = spread.reshape(
            npad, K
        ).astype(np.int32)
        w_pad[c * NO_CORE : c * NO_CORE + NO_REAL] = w[c * NO_REAL : (c + 1) * NO_REAL]
    cores_idx = [idx_pad[c * NO_CORE : (c + 1) * NO_CORE] for c in range(8)]
    cores_w = [w_pad[c * NO_CORE : (c + 1) * NO_CORE] for c in range(8)]
    assigns = [_balance_subs(cores_idx[c]) for c in range(8)]

    nws = np.stack([_bin_sizes(cores_idx[c], assigns[c]) for c in range(8)])
    pbv = int(max(_ceil_to(int(nws.max()), 2), 16))
    assert (NS * pbv) % 16 == 0, pbv
    caps = np.full(NW, pbv, dtype=np.int64)
    coff = np.concatenate([[0], np.cumsum(caps)])[:NW]
    cw = int(caps.sum())
    assert cw <= 32768, cw

    preps = [
        _prep_core(cores_idx[c], cores_w[c], caps, coff, assigns[c])
        for c in range(8)
    ]

    key = (pbv,)
    if key not in _CACHE:
        _CACHE.clear()
        _CACHE[key] = _build_nc(caps, coff, cw)
    nc = _CACHE[key]

    ni1 = NS * pbv
    in_maps = []
    for c in range(8):
        p = preps[c]
        s1i = np.zeros((NR, 128, ni1 // 16), dtype=np.int16)
        for wv in range(NW):
            r, u = divmod(wv, 8)
            s1i[r, 16 * u : 16 * u + 16, :] = _wrap16(p["s1"][wv])
        s2i = np.zeros((S2R, 128, NI2 // 16), dtype=np.int16)
        wrr = np.zeros((S2R, 128, NI2 * 2), dtype=np.float16)
        for r2 in range(S2R):
            for u in range(8):
                s2i[r2, 16 * u : 16 * u + 16, :] = _wrap16(p["s2i"][r2, u])
                wrr[r2, 16 * u : 16 * u + 16, :] = np.repeat(
                    p["wrep"][r2, u], 2
                ).astype(np.float16)[None, :]
        in_maps.append({"xg": xpk, "s1i": s1i, "s2i": s2i, "wr": wrr})

    res = run_bass_kernel_spmd(nc, in_maps, core_ids=list(range(8)))
    kernel._last_exec_ns = res.exec_time_ns
    y = np.zeros((B, N_OUT), dtype=np.float32)
    for c in range(8):
        ydev = res.results[c]["y"]                    # [16, NS*SUB] f32 pairs
        yh = ydev.view(np.float16).reshape(16, NS * SUB, 2)
        ycore = np.empty((B, NO_CORE), dtype=np.float32)
        for s, outs in enumerate(preps[c]["outs_of_sub"]):
            blk = yh[:, s * SUB : s * SUB + outs.size, :]   # [16, n, 2]
            ycore[0::2, outs] = blk[:, :, 0].astype(np.float32)
            ycore[1::2, outs] = blk[:, :, 1].astype(np.float32)
        y[:, c * NO_REAL : (c + 1) * NO_REAL] = ycore[:, :NO_REAL]
    return y
